# revision 2
# baseline (speedup 1.0000x reference)
"""BP-MLL loss kernel for Trainium2 (Bass/Tile), data-parallel over 8 NeuronCores.

Reference computation (per row r of [B, L] inputs):
    s_pos[r] = sum_{j: t=1} exp(-x[r,j])
    s_neg[r] = sum_{j: t=0} exp( x[r,j])
    n_pos[r] = #{j: t=1},  n_neg[r] = L - n_pos[r]
    loss     = sum_r s_pos[r]*s_neg[r] / (n_pos[r]*n_neg[r])

Sharding: batch dim B=8192 split 8 ways (1024 rows/core); each core computes a
scalar partial loss on-device; host sums the 8 partials.

The kernel is HBM-bound: 81.92 MB/core (x f32 + t i32) against a ~420 GB/s
16-engine SDMA array. A single HWDGE queue cannot reliably keep all 16 engines
fed (descriptor dispatch per queue is a secondary limit), so x loads issue on
the SP queue and t loads on the Activation queue; measured back-to-back this is
255us (one queue) vs 206us (two queues) for the bare stream.

Compute uses ONE exp pass (not two) so the Activation engine has headroom to
issue the t DMAs. With u = C*t - x, C=64 (exp(+-C) finite in f32):
    ACT:  e = exp(-u)              accum -> S1 = s_neg + e^-C*sum_{t=1}e^x
    ACT:  identity(x)              accum -> SX = sum(x)
    DVE:  u (scalar_tensor_tensor) accum -> SU = C*n_pos - sum(x)
    DVE:  r = 1/e  (reciprocal)    then tensor_reduce -> S2 = e^C*s_pos + eps
The contamination terms are ~1e-24 relative (e^-64 scaled) -- negligible.
n_pos is recovered exactly from V = SU + SX = C*n_pos (C a power of two,
V ~ 3.2e5 carries < 0.3 absolute error; relative error in n_pos*n_neg ~2e-6).
exp/identity share one ACT table set, so no ACT table reloads occur.

Per row group the slot partials reduce to S1r,S2r,SUr,SXr; then
    denom = (V - C*L)*V = -C^2 * n_pos*n_neg
    contrib = S1r*S2r / denom
    ps += w^T contrib   (PSUM matmul, w = -C^2*e^-C, accumulated over rgs)
which telescopes to the exact per-core partial loss.

The last row group's chunks taper (2500,2500,1250x4) so the post-stream drain
is one small chunk's pipeline (~4us) instead of a full chunk's.
"""

import math

import numpy as np

import concourse.bacc as bacc
import concourse.bass as bass
import concourse.tile as tile
from concourse import mybir
from concourse.bass_utils import run_bass_kernel_spmd

F32 = mybir.dt.float32
I32 = mybir.dt.int32
AF = mybir.ActivationFunctionType
ALU = mybir.AluOpType

B, L = 8192, 10000
N_CORES = 8
ROWS = B // N_CORES  # rows per core
P = 128
C = 64.0  # mask scale: power of 2; exp(+-C) finite in f32, e^-C ~ 1.6e-28


def build_bass(
    rows=ROWS,
    cols=L,
    f_c=2500,
    io_bufs=6,
    u_bufs=3,
    e_bufs=2,
    last_rg=(2500, 2500, 1250, 1250, 1250, 1250),  # tapered final row group
    split_t=True,  # t loads on the Activation HWDGE queue (2nd queue)
    dma_only=False,
):
    """Build the per-core Bass program. Same program runs SPMD on all cores."""
    assert rows % P == 0 and cols % f_c == 0
    n_rg = rows // P
    n_ch = cols // f_c
    if last_rg is not None:
        assert sum(last_rg) == cols

    widths = [f_c] * n_ch
    last_widths = list(last_rg) if last_rg else widths

    def chunks_for(rg):
        ws = last_widths if rg == n_rg - 1 else widths
        offs = np.concatenate([[0], np.cumsum(ws)[:-1]]).tolist()
        return list(zip(offs, ws))

    n_slots = sum(len(chunks_for(rg)) for rg in range(n_rg))

    nc = bacc.Bacc("TRN2", target_bir_lowering=False, debug=False)
    x = nc.dram_tensor("x", [rows, cols], F32, kind="ExternalInput").ap()
    t = nc.dram_tensor("t", [rows, cols], I32, kind="ExternalInput").ap()
    out = nc.dram_tensor("out", [1, 1], F32, kind="ExternalOutput").ap()

    with tile.TileContext(nc) as tc:
        with (
            tc.tile_pool(name="io", bufs=io_bufs) as io_pool,
            tc.tile_pool(name="upool", bufs=u_bufs) as u_pool,
            tc.tile_pool(name="epool", bufs=e_bufs) as e_pool,
            tc.tile_pool(name="scr", bufs=1) as scr_pool,
            tc.tile_pool(name="acc", bufs=1) as acc_pool,
            tc.tile_pool(name="small", bufs=1) as small_pool,
            tc.tile_pool(name="psum", bufs=1, space="PSUM") as psum_pool,
        ):
            acc_s1 = acc_pool.tile([P, n_slots], F32, tag="acc_s1")
            acc_s2 = acc_pool.tile([P, n_slots], F32, tag="acc_s2")
            acc_su = acc_pool.tile([P, n_slots], F32, tag="acc_su")
            acc_sx = acc_pool.tile([P, n_slots], F32, tag="acc_sx")

            # dead-store sinks for ACT identity / DVE reciprocal outputs
            scr_id = scr_pool.tile([P, f_c], F32, tag="scr_id")
            scr_r = scr_pool.tile([P, f_c], F32, tag="scr_r")

            if not dma_only:
                w = acc_pool.tile([P, 1], F32, tag="w")
                nc.vector.memset(w[:], -(C * C) * math.exp(-C))
                ps = psum_pool.tile([1, 1], F32, tag="ps")

            sl = 0
            for rg in range(n_rg):
                r0 = rg * P
                rg_chunks = chunks_for(rg)
                s0 = sl
                pending = None  # (e_tile, width, slot) awaiting recip+reduce
                for c0, fw in rg_chunks:
                    xt = io_pool.tile([P, fw], F32, tag="x")
                    tt = io_pool.tile([P, fw], I32, tag="t")
                    nc.sync.dma_start(xt[:], x[r0 : r0 + P, c0 : c0 + fw])
                    t_eng = nc.scalar if split_t else nc.sync
                    t_eng.dma_start(tt[:], t[r0 : r0 + P, c0 : c0 + fw])
                    if dma_only:
                        sl += 1
                        continue

                    # ACT: SX += sum(x)  (same table set as Exp -> no reload)
                    nc.scalar.activation(
                        scr_id[:, 0:fw],
                        xt[:],
                        AF.Identity,
                        accum_out=acc_sx[:, sl : sl + 1],
                    )
                    ut = u_pool.tile([P, fw], F32, tag="u")
                    # DVE: u = C*t - x ; SU += sum(u) = C*n_pos - sum(x)
                    nc.vector.scalar_tensor_tensor(
                        ut[:],
                        tt[:],
                        C,
                        xt[:],
                        op0=ALU.mult,
                        op1=ALU.subtract,
                        accum_out=acc_su[:, sl : sl + 1],
                    )
                    et = e_pool.tile([P, fw], F32, tag="e")
                    # ACT: e = exp(-u); S1 += sum(e) ~= s_neg
                    nc.scalar.activation(
                        et[:],
                        ut[:],
                        AF.Exp,
                        scale=-1.0,
                        accum_out=acc_s1[:, sl : sl + 1],
                    )
                    # DVE recip+reduce of the PREVIOUS chunk (keeps stt ahead
                    # of the cross-engine e dependency)
                    if pending is not None:
                        pe, pw, psl = pending
                        nc.vector.reciprocal(scr_r[:, 0:pw], pe[:])
                        nc.vector.tensor_reduce(
                            acc_s2[:, psl : psl + 1],
                            scr_r[:, 0:pw],
                            axis=mybir.AxisListType.X,
                            op=ALU.add,
                        )
                    pending = (et, fw, sl)
                    sl += 1

                if dma_only:
                    continue

                # flush the last chunk's recip+reduce
                pe, pw, psl = pending
                nc.vector.reciprocal(scr_r[:, 0:pw], pe[:])
                nc.vector.tensor_reduce(
                    acc_s2[:, psl : psl + 1],
                    scr_r[:, 0:pw],
                    axis=mybir.AxisListType.X,
                    op=ALU.add,
                )

                # --- per-row-group epilogue (overlaps later chunks' stream) ---
                s1 = sl
                S1r = small_pool.tile([P, 1], F32, tag="S1r")
                S2r = small_pool.tile([P, 1], F32, tag="S2r")
                SUr = small_pool.tile([P, 1], F32, tag="SUr")
                SXr = small_pool.tile([P, 1], F32, tag="SXr")
                for dst, src in (
                    (S1r, acc_s1),
                    (S2r, acc_s2),
                    (SUr, acc_su),
                    (SXr, acc_sx),
                ):
                    nc.vector.tensor_reduce(
                        dst[:],
                        src[:, s0:s1],
                        axis=mybir.AxisListType.X,
                        op=ALU.add,
                    )
                V = small_pool.tile([P, 1], F32, tag="V")
                nc.vector.tensor_tensor(V[:], SUr[:], SXr[:], op=ALU.add)
                # denom = (V - C*L) * V = -C^2 * n_pos * n_neg   (V = C*n_pos)
                denom = small_pool.tile([P, 1], F32, tag="denom")
                nc.vector.scalar_tensor_tensor(
                    denom[:],
                    V[:],
                    C * float(cols),
                    V[:],
                    op0=ALU.subtract,
                    op1=ALU.mult,
                )
                numer = small_pool.tile([P, 1], F32, tag="numer")
                nc.vector.tensor_tensor(numer[:], S1r[:], S2r[:], op=ALU.mult)
                recip = small_pool.tile([P, 1], F32, tag="recip")
                nc.vector.reciprocal(recip[:], denom[:])
                contrib = small_pool.tile([P, 1], F32, tag="contrib")
                nc.vector.tensor_tensor(
                    contrib[:], numer[:], recip[:], op=ALU.mult
                )
                # ps += w^T @ contrib ; w = -C^2*e^-C folds every scale factor
                nc.tensor.matmul(
                    ps[:],
                    w[:],
                    contrib[:],
                    start=(rg == 0),
                    stop=(rg == n_rg - 1),
                )

            res = small_pool.tile([1, 1], F32, tag="res")
            if dma_only:
                nc.vector.memset(res[:], 0.0)
            else:
                nc.vector.tensor_copy(res[:], ps[:])
            nc.sync.dma_start(out[0:1, 0:1], res[:])

    nc.compile()
    return nc


_NC_CACHE = {}


def _get_nc():
    if "nc" not in _NC_CACHE:
        _NC_CACHE["nc"] = build_bass()
    return _NC_CACHE["nc"]


def kernel(input, target):
    x = np.ascontiguousarray(np.asarray(input, dtype=np.float32))
    t = np.ascontiguousarray(np.asarray(target, dtype=np.int32))
    assert x.shape == (B, L) and t.shape == (B, L)

    nc = _get_nc()
    in_maps = [
        {
            "x": x[i * ROWS : (i + 1) * ROWS],
            "t": t[i * ROWS : (i + 1) * ROWS],
        }
        for i in range(N_CORES)
    ]
    res = run_bass_kernel_spmd(nc, in_maps, core_ids=list(range(N_CORES)))
    partials = np.array(
        [res.results[i]["out"][0, 0] for i in range(N_CORES)], dtype=np.float64
    )
    return np.float32(partials.sum())


# revision 6
# speedup vs baseline: 2.7448x; 2.7448x over previous
"""BP-MLL loss kernel for Trainium2 (Bass/Tile), data-parallel over 8 NeuronCores.

Reference computation (per row r of [B, L] inputs):
    s_pos[r] = sum_{j: t=1} exp(-x[r,j])
    s_neg[r] = sum_{j: t=0} exp( x[r,j])
    n_pos[r] = #{j: t=1},  n_neg[r] = L - n_pos[r]
    loss     = sum_r s_pos[r]*s_neg[r] / (n_pos[r]*n_neg[r])

Sharding: batch dim B=8192 split 8 ways (1024 rows/core); each core computes a
scalar partial loss on-device; host sums the 8 partials.

The kernel is HBM-bound: 81.92 MB/core (x f32 + t i32) against a ~420 GB/s
16-engine SDMA array. A single HWDGE queue cannot reliably keep all 16 engines
fed (descriptor dispatch per queue is a secondary limit), so x loads issue on
the SP queue and t loads on the Activation queue; measured back-to-back this is
255us (one queue) vs 206us (two queues) for the bare stream.

Compute: with u = C*t - x, C=64 (exp(+-C) finite in f32, so unlike the
C=8192 flush-to-zero trick, BOTH exp directions stay finite):
    ACT:  exp(-u)                  accum -> S1 = s_neg + e^-C*sum_{t=1}e^x
    ACT:  exp(+u)                  accum -> S2 = e^C*s_pos + sum_{t=0}e^-x
    DVE:  u (scalar_tensor_tensor) accum -> SU = C*n_pos - sum(x)
    DVE:  tensor_reduce(x)               -> SX = sum(x)
The contamination terms are ~1e-24 relative (e^-C scaled) -- negligible.
n_pos is recovered exactly from V = SU + SX = C*n_pos (C a power of two,
V ~ 3.2e5 carries < 0.3 absolute error; relative error in n_pos*n_neg ~2e-6).
Both ACT passes use the same Exp table -> no ACT table reloads. (A DVE
reciprocal variant of S2 was 8 cycles/elem -- 3.3x slower overall; and a
third ACT pass would blow the 6.15us/chunk ACT budget, so SX rides on DVE.)

Per row group the slot partials reduce to S1r,S2r,SUr,SXr; then
    denom = (V - C*L)*V = -C^2 * n_pos*n_neg
    contrib = S1r*S2r / denom
    ps += w^T contrib   (PSUM matmul, w = -C^2*e^-C, accumulated over rgs)
which telescopes to the exact per-core partial loss.

The last row group's chunks taper (2500,2500,1250x4) so the post-stream drain
is one small chunk's pipeline (~4us) instead of a full chunk's.
"""

import math

import numpy as np

import concourse.bacc as bacc
import concourse.bass as bass
import concourse.tile as tile
from concourse import mybir
from concourse.bass_utils import run_bass_kernel_spmd

F32 = mybir.dt.float32
I32 = mybir.dt.int32
AF = mybir.ActivationFunctionType
ALU = mybir.AluOpType

B, L = 8192, 10000
N_CORES = 8
ROWS = B // N_CORES  # rows per core
P = 128
C = 64.0  # mask scale: power of 2; exp(+-C) finite in f32, e^-C ~ 1.6e-28


def build_bass(
    rows=ROWS,
    cols=L,
    f_c=2500,
    io_bufs=6,
    u_bufs=3,
    e_bufs=2,
    last_rg=(2500, 2500, 1250, 1250, 1250, 1250),  # tapered final row group
    split_t=True,  # t loads on the Activation HWDGE queue (2nd queue)
    dma_only=False,
):
    """Build the per-core Bass program. Same program runs SPMD on all cores."""
    assert rows % P == 0 and cols % f_c == 0
    n_rg = rows // P
    n_ch = cols // f_c
    if last_rg is not None:
        assert sum(last_rg) == cols

    widths = [f_c] * n_ch
    last_widths = list(last_rg) if last_rg else widths

    def chunks_for(rg):
        ws = last_widths if rg == n_rg - 1 else widths
        offs = np.concatenate([[0], np.cumsum(ws)[:-1]]).tolist()
        return list(zip(offs, ws))

    n_slots = sum(len(chunks_for(rg)) for rg in range(n_rg))

    nc = bacc.Bacc("TRN2", target_bir_lowering=False, debug=False)
    x = nc.dram_tensor("x", [rows, cols], F32, kind="ExternalInput").ap()
    t = nc.dram_tensor("t", [rows, cols], I32, kind="ExternalInput").ap()
    out = nc.dram_tensor("out", [1, 1], F32, kind="ExternalOutput").ap()

    with tile.TileContext(nc) as tc:
        with (
            tc.tile_pool(name="io", bufs=io_bufs) as io_pool,
            tc.tile_pool(name="upool", bufs=u_bufs) as u_pool,
            tc.tile_pool(name="scr", bufs=1) as scr_pool,
            tc.tile_pool(name="acc", bufs=1) as acc_pool,
            tc.tile_pool(name="small", bufs=1) as small_pool,
            tc.tile_pool(name="psum", bufs=1, space="PSUM") as psum_pool,
        ):
            acc_s1 = acc_pool.tile([P, n_slots], F32, tag="acc_s1")
            acc_s2 = acc_pool.tile([P, n_slots], F32, tag="acc_s2")
            acc_su = acc_pool.tile([P, n_slots], F32, tag="acc_su")
            acc_sx = acc_pool.tile([P, n_slots], F32, tag="acc_sx")

            # dead-store sinks for the two ACT exp outputs (only the
            # accumulators matter; same-engine WAW keeps these race-free)
            scr_e1 = scr_pool.tile([P, f_c], F32, tag="scr_e1")
            scr_e2 = scr_pool.tile([P, f_c], F32, tag="scr_e2")

            if not dma_only:
                w = acc_pool.tile([P, 1], F32, tag="w")
                nc.vector.memset(w[:], -(C * C) * math.exp(-C))
                ps = psum_pool.tile([1, 1], F32, tag="ps")

            sl = 0
            for rg in range(n_rg):
                r0 = rg * P
                rg_chunks = chunks_for(rg)
                s0 = sl
                for c0, fw in rg_chunks:
                    xt = io_pool.tile([P, fw], F32, tag="x")
                    tt = io_pool.tile([P, fw], I32, tag="t")
                    nc.sync.dma_start(xt[:], x[r0 : r0 + P, c0 : c0 + fw])
                    t_eng = nc.scalar if split_t else nc.sync
                    t_eng.dma_start(tt[:], t[r0 : r0 + P, c0 : c0 + fw])
                    if dma_only:
                        sl += 1
                        continue

                    ut = u_pool.tile([P, fw], F32, tag="u")
                    # DVE: u = C*t - x ; SU += sum(u) = C*n_pos - sum(x)
                    nc.vector.scalar_tensor_tensor(
                        ut[:],
                        tt[:],
                        C,
                        xt[:],
                        op0=ALU.mult,
                        op1=ALU.subtract,
                        accum_out=acc_su[:, sl : sl + 1],
                    )
                    # DVE: SX = sum(x)  (x tile is hot; frees ACT for 2 exps)
                    nc.vector.tensor_reduce(
                        acc_sx[:, sl : sl + 1],
                        xt[:],
                        axis=mybir.AxisListType.X,
                        op=ALU.add,
                    )
                    # ACT: exp(-u): t=0 -> e^x  ; S1 += sum ~= s_neg
                    nc.scalar.activation(
                        scr_e1[:, 0:fw],
                        ut[:],
                        AF.Exp,
                        scale=-1.0,
                        accum_out=acc_s1[:, sl : sl + 1],
                    )
                    # ACT: exp(+u): t=1 -> e^C*e^-x ; S2 += sum = e^C*s_pos
                    nc.scalar.activation(
                        scr_e2[:, 0:fw],
                        ut[:],
                        AF.Exp,
                        scale=1.0,
                        accum_out=acc_s2[:, sl : sl + 1],
                    )
                    sl += 1

                if dma_only:
                    continue

                # --- per-row-group epilogue (overlaps later chunks' stream) ---
                s1 = sl
                S1r = small_pool.tile([P, 1], F32, tag="S1r")
                S2r = small_pool.tile([P, 1], F32, tag="S2r")
                SUr = small_pool.tile([P, 1], F32, tag="SUr")
                SXr = small_pool.tile([P, 1], F32, tag="SXr")
                for dst, src in (
                    (S1r, acc_s1),
                    (S2r, acc_s2),
                    (SUr, acc_su),
                    (SXr, acc_sx),
                ):
                    nc.vector.tensor_reduce(
                        dst[:],
                        src[:, s0:s1],
                        axis=mybir.AxisListType.X,
                        op=ALU.add,
                    )
                V = small_pool.tile([P, 1], F32, tag="V")
                nc.vector.tensor_tensor(V[:], SUr[:], SXr[:], op=ALU.add)
                # denom = (V - C*L) * V = -C^2 * n_pos * n_neg   (V = C*n_pos)
                denom = small_pool.tile([P, 1], F32, tag="denom")
                nc.vector.scalar_tensor_tensor(
                    denom[:],
                    V[:],
                    C * float(cols),
                    V[:],
                    op0=ALU.subtract,
                    op1=ALU.mult,
                )
                numer = small_pool.tile([P, 1], F32, tag="numer")
                nc.vector.tensor_tensor(numer[:], S1r[:], S2r[:], op=ALU.mult)
                recip = small_pool.tile([P, 1], F32, tag="recip")
                nc.vector.reciprocal(recip[:], denom[:])
                contrib = small_pool.tile([P, 1], F32, tag="contrib")
                nc.vector.tensor_tensor(
                    contrib[:], numer[:], recip[:], op=ALU.mult
                )
                # ps += w^T @ contrib ; w = -C^2*e^-C folds every scale factor
                nc.tensor.matmul(
                    ps[:],
                    w[:],
                    contrib[:],
                    start=(rg == 0),
                    stop=(rg == n_rg - 1),
                )

            res = small_pool.tile([1, 1], F32, tag="res")
            if dma_only:
                nc.vector.memset(res[:], 0.0)
            else:
                nc.vector.tensor_copy(res[:], ps[:])
            nc.sync.dma_start(out[0:1, 0:1], res[:])

    nc.compile()
    return nc


_NC_CACHE = {}


def _get_nc():
    if "nc" not in _NC_CACHE:
        _NC_CACHE["nc"] = build_bass()
    return _NC_CACHE["nc"]


def kernel(input, target):
    x = np.ascontiguousarray(np.asarray(input, dtype=np.float32))
    t = np.ascontiguousarray(np.asarray(target, dtype=np.int32))
    assert x.shape == (B, L) and t.shape == (B, L)

    nc = _get_nc()
    in_maps = [
        {
            "x": x[i * ROWS : (i + 1) * ROWS],
            "t": t[i * ROWS : (i + 1) * ROWS],
        }
        for i in range(N_CORES)
    ]
    res = run_bass_kernel_spmd(nc, in_maps, core_ids=list(range(N_CORES)))
    partials = np.array(
        [res.results[i]["out"][0, 0] for i in range(N_CORES)], dtype=np.float64
    )
    return np.float32(partials.sum())
